# revision 5
# baseline (speedup 1.0000x reference)
"""Trainium2 Bass kernel for nn_ConsolidationModel.

Mathematical reduction (verified bit-exact against the reference scan):
the scan's control flow is data-independent (count depends only on t).
Consolidation fires at t=15/31/47, but between consecutive firings the
8-slot FIFO buffer receives 4 appends + 12 shift-appends, which evicts
every consolidated row (4) plus everything else before the next firing
— and after the last firing (t=47) there are 4 appends + 11 shifts, so
by t=62 the buffer holds exactly the embeddings of tokens 55..62 with
count=8.  The final output is therefore:

    mem  = mean_p embed[seqs[:, 55+p]]          (p = 0..7)
    h    = concat([embed[query_tok], mem], -1)  (B, 128)
    out  = relu(h @ r1_w.T + r1_b) @ r2_w.T + r2_b

On device (per core, batch shard of 256 rows) the token gather is done
as a one-hot/histogram matmul against the 64x64 embed table:
  hist[b, v]  = #{p : seqs[b, 55+p] == v}  -> built with iota==token
                compares on the vector engine, transposed on the PE
  mem_sum^T   = embed^T @ hist^T            (the /8 is folded into r1_w)
  qemb^T      = embed^T @ onehot(query)^T
  hidden^T    = relu(r1_w @ h^T + r1_b)
  logits^T    = r2_w @ hidden^T + r2_b      -> PE-transposed back to
                (256, 64) natural layout and DMA'd out.

Sharding: pure data parallel over batch across 8 cores; the tiny
parameters are replicated.
"""

import numpy as np

N_CORES = 8
B = 2048           # full batch
BS = B // N_CORES  # 256 per-core batch shard
H = 64             # hidden dim
V = 64             # vocab
TAIL_LO, TAIL_HI = 55, 63  # token positions that survive in the buffer
NPOS = TAIL_HI - TAIL_LO   # 8
P = 128            # SBUF partitions

_compiled_nc = None


def _build_program():
    import concourse.bacc as bacc
    import concourse.mybir as mybir
    from concourse import tile

    f32 = mybir.dt.float32
    eq = mybir.AluOpType.is_equal
    Relu = mybir.ActivationFunctionType.Relu

    nc = bacc.Bacc("TRN2", target_bir_lowering=False, debug=False,
                   num_devices=N_CORES)

    toks_d = nc.declare_dram_parameter("toks", [BS, NPOS + 1], f32, isOutput=False)
    iota_d = nc.declare_dram_parameter("iota", [P, V], f32, isOutput=False)
    embed_d = nc.declare_dram_parameter("embed", [V, H], f32, isOutput=False)
    r1wT_d = nc.declare_dram_parameter("r1wT", [2 * H, H], f32, isOutput=False)
    r1b_d = nc.declare_dram_parameter("r1b", [H, 1], f32, isOutput=False)
    r2wT_d = nc.declare_dram_parameter("r2wT", [H, V], f32, isOutput=False)
    r2b_d = nc.declare_dram_parameter("r2b", [V, 1], f32, isOutput=False)
    ident_d = nc.declare_dram_parameter("ident", [P, P], f32, isOutput=False)
    out_d = nc.declare_dram_parameter("out", [BS, V], f32, isOutput=True)

    with tile.TileContext(nc) as tc:
        with (
            tc.tile_pool(name="const", bufs=1) as cpool,
            tc.tile_pool(name="work", bufs=2) as wpool,
            tc.tile_pool(name="ps", bufs=1, space="PSUM") as pp,
            tc.tile_pool(name="pst", bufs=2, space="PSUM") as ppt,
        ):
            iota_sb = cpool.tile([P, V], f32)
            nc.sync.dma_start(iota_sb[:], iota_d[:])
            id_sb = cpool.tile([P, P], f32)
            nc.sync.dma_start(id_sb[:], ident_d[:])
            embed_sb = cpool.tile([V, H], f32)
            nc.sync.dma_start(embed_sb[:], embed_d[:])
            r1wT_sb = cpool.tile([2 * H, H], f32)
            nc.sync.dma_start(r1wT_sb[:], r1wT_d[:])
            r1b_sb = cpool.tile([H, 1], f32)
            nc.sync.dma_start(r1b_sb[:], r1b_d[:])
            r2wT_sb = cpool.tile([H, V], f32)
            nc.sync.dma_start(r2wT_sb[:], r2wT_d[:])
            r2b_sb = cpool.tile([V, 1], f32)
            nc.sync.dma_start(r2b_sb[:], r2b_d[:])

            histT_sb = cpool.tile([V, BS], f32)  # vocab on partitions
            qT_sb = cpool.tile([V, BS], f32)

            for i in range(BS // P):  # 2 batch tiles of 128
                toks_sb = wpool.tile([P, NPOS + 1], f32)
                nc.sync.dma_start(toks_sb[:], toks_d[i * P:(i + 1) * P, :])

                # hist[b, v] = sum_p (toks[b, p] == v); batch on partitions
                hist = wpool.tile([P, V], f32)
                nc.vector.tensor_scalar(hist[:], iota_sb[:], toks_sb[:, 0:1], None, eq)
                for p in range(1, NPOS):
                    msk = wpool.tile([P, V], f32)
                    nc.vector.tensor_scalar(msk[:], iota_sb[:], toks_sb[:, p:p + 1], None, eq)
                    nc.vector.tensor_add(hist[:], hist[:], msk[:])
                qm = wpool.tile([P, V], f32)
                nc.vector.tensor_scalar(qm[:], iota_sb[:], toks_sb[:, NPOS:NPOS + 1], None, eq)

                # transpose to vocab-on-partitions and collect into (V, BS)
                tp = ppt.tile([V, P], f32, tag="tp")
                nc.tensor.transpose(tp[:], hist[:], id_sb[:])
                nc.vector.tensor_copy(histT_sb[:, i * P:(i + 1) * P], tp[:])
                tq = ppt.tile([V, P], f32, tag="tp")
                nc.tensor.transpose(tq[:], qm[:], id_sb[:])
                nc.vector.tensor_copy(qT_sb[:, i * P:(i + 1) * P], tq[:])

            # mem_sum^T = embed^T @ hist^T ; qemb^T = embed^T @ onehot^T
            memT_ps = pp.tile([H, BS], f32, tag="memT")
            nc.tensor.matmul(memT_ps[:], embed_sb[:], histT_sb[:], start=True, stop=True)
            qembT_ps = pp.tile([H, BS], f32, tag="qembT")
            nc.tensor.matmul(qembT_ps[:], embed_sb[:], qT_sb[:], start=True, stop=True)

            # h^T = [qemb^T ; mem_sum^T]  (the /8 is folded into r1wT rows 64:128)
            hT_sb = cpool.tile([2 * H, BS], f32)
            nc.vector.tensor_copy(hT_sb[0:H, :], qembT_ps[:])
            nc.vector.tensor_copy(hT_sb[H:2 * H, :], memT_ps[:])

            hid_ps = pp.tile([H, BS], f32, tag="hid")
            nc.tensor.matmul(hid_ps[:], r1wT_sb[:], hT_sb[:], start=True, stop=True)
            hid_sb = cpool.tile([H, BS], f32)
            nc.scalar.activation(hid_sb[:], hid_ps[:], Relu, bias=r1b_sb[:, 0:1])

            log_ps = pp.tile([V, BS], f32, tag="log")
            nc.tensor.matmul(log_ps[:], r2wT_sb[:], hid_sb[:], start=True, stop=True)
            log_sb = cpool.tile([V, BS], f32)
            nc.scalar.add(log_sb[:], log_ps[:], r2b_sb[:, 0:1])

            # transpose back to (BS, V) natural layout and store
            for i in range(BS // P):
                op = ppt.tile([P, V], f32, tag="tp")
                nc.tensor.transpose(op[:], log_sb[:, i * P:(i + 1) * P], id_sb[0:V, 0:V])
                ob = wpool.tile([P, V], f32, tag="outb")
                nc.vector.tensor_copy(ob[:], op[:])
                nc.sync.dma_start(out_d[i * P:(i + 1) * P, :], ob[:])

    nc.compile()
    return nc


def _prep_in_maps(inputs):
    embed = np.asarray(inputs["embed"], dtype=np.float32)[:V]      # (64, 64)
    r1_w = np.asarray(inputs["r1_w"], dtype=np.float32)            # (64, 128)
    r1_b = np.asarray(inputs["r1_b"], dtype=np.float32)            # (64,)
    r2_w = np.asarray(inputs["r2_w"], dtype=np.float32)            # (64, 64)
    r2_b = np.asarray(inputs["r2_b"], dtype=np.float32)            # (64,)
    seqs = np.asarray(inputs["seqs"])                              # (B, 64) int
    query = np.asarray(inputs["query_tok"])                        # (B,) int

    r1wT = np.ascontiguousarray(r1_w.T)                            # (128, 64)
    r1wT[H:2 * H, :] *= np.float32(1.0 / NPOS)                     # fold mean /8
    r2wT = np.ascontiguousarray(r2_w.T)                            # (64, 64)

    toks = np.empty((B, NPOS + 1), np.float32)
    toks[:, :NPOS] = seqs[:, TAIL_LO:TAIL_HI]
    toks[:, NPOS] = query

    iota_tile = np.ascontiguousarray(
        np.broadcast_to(np.arange(V, dtype=np.float32), (P, V)))
    ident = np.eye(P, dtype=np.float32)
    r1b_col = np.ascontiguousarray(r1_b.reshape(H, 1))
    r2b_col = np.ascontiguousarray(r2_b.reshape(V, 1))

    shared = {
        "iota": iota_tile, "embed": np.ascontiguousarray(embed),
        "r1wT": r1wT, "r1b": r1b_col, "r2wT": r2wT, "r2b": r2b_col,
        "ident": ident,
    }
    return [
        {"toks": np.ascontiguousarray(toks[c * BS:(c + 1) * BS]), **shared}
        for c in range(N_CORES)
    ]


def kernel(**inputs):
    global _compiled_nc
    from concourse.bass_utils import run_bass_kernel_spmd

    in_maps = _prep_in_maps(inputs)
    if _compiled_nc is None:
        _compiled_nc = _build_program()
    res = run_bass_kernel_spmd(_compiled_nc, in_maps, list(range(N_CORES)))
    out = np.concatenate([res.results[c]["out"] for c in range(N_CORES)], axis=0)
    return out.astype(np.float32, copy=False)


if __name__ == "__main__":
    rng = np.random.default_rng(0)
    demo = {
        "embed": rng.standard_normal((V + 2, H)).astype(np.float32),
        "r1_w": rng.standard_normal((H, 2 * H)).astype(np.float32) * 0.05,
        "r1_b": rng.standard_normal(H).astype(np.float32) * 0.02,
        "r2_w": rng.standard_normal((V, H)).astype(np.float32) * 0.05,
        "r2_b": rng.standard_normal(V).astype(np.float32) * 0.02,
        "seqs": rng.integers(0, V, (B, 64)),
        "query_tok": rng.integers(0, V, (B,)),
    }
    out = kernel(**demo)
    tail = demo["embed"][demo["seqs"][:, TAIL_LO:TAIL_HI]]
    mem = tail.sum(1) / NPOS
    h = np.concatenate([demo["embed"][demo["query_tok"]], mem], -1)
    exp = np.maximum(h @ demo["r1_w"].T + demo["r1_b"], 0) @ demo["r2_w"].T + demo["r2_b"]
    err = np.abs(out - exp).max() / np.abs(exp).max()
    print("self-check rel err:", err)


# revision 6
# speedup vs baseline: 1.5345x; 1.5345x over previous
"""Trainium2 Bass kernel for nn_ConsolidationModel.

Mathematical reduction (verified bit-exact against the reference scan in
float64/numpy): the scan's control flow is data-independent (count
depends only on t).  Consolidation fires at t=15/31/47, but between
consecutive firings the 8-slot FIFO receives 4 appends + 12
shift-appends, which evicts every consolidated row before the next
firing — and after the last firing (t=47) there are 4 appends + 11
shifts, so at t=62 the buffer holds exactly the embeddings of tokens
55..62 with count=8.  The model output is therefore:

    mem  = mean_p embed[seqs[:, 55+p]]          (p = 0..7)
    h    = concat([embed[query_tok], mem], -1)  (B, 128)
    out  = relu(h @ r1_w.T + r1_b) @ r2_w.T + r2_b

Device algorithm (per core, batch shard of 256 rows, all in the
"transposed" layout with vocab/hidden on partitions and batch on the
free axis):

  wide  (64, 2304) <- one DMA with a stride-0 partition AP: the 9
                      token rows (8 tail positions + query) broadcast
                      across the 64 vocab partitions
  masks = (wide == iota column)   one-hot masks, built by 2 DVE ops
  histT (64, 256) = bf16 add-tree over the 8 position masks
  hidT  (64, 256) = B^T @ histT + [A; r1_b]^T @ [qmaskT; ones]
                      where A = embed @ r1_w[:, :64].T and
                      B = embed @ r1_w[:, 64:].T / 8 are folded on the
                      host (weight preprocessing; data-independent)
  hid   = relu(hidT)
  logT  (64, 256) = [r2_w.T; r2_b]^T @ [hid; ones]
  -> DMA out; the host transposes each shard back to (256, 64) while
     gathering the 8 shards.

Sharding: pure data parallel over batch across 8 cores; parameters
replicated.
"""

import numpy as np

N_CORES = 8
B = 2048           # full batch
BS = B // N_CORES  # 256 per-core batch shard
H = 64             # hidden dim
V = 64             # vocab
TAIL_LO, TAIL_HI = 55, 63  # token positions that survive in the buffer
NPOS = TAIL_HI - TAIL_LO   # 8

_compiled_nc = None


def _build_program():
    import concourse.bacc as bacc
    import concourse.mybir as mybir
    from concourse import tile

    f32 = mybir.dt.float32
    bf16 = mybir.dt.bfloat16
    eq = mybir.AluOpType.is_equal

    nc = bacc.Bacc("TRN2", target_bir_lowering=False, debug=False,
                   num_devices=N_CORES)

    toks_d = nc.declare_dram_parameter("toks", [1, 9 * BS], f32, isOutput=False)
    cst_d = nc.declare_dram_parameter("cst", [H + 1, 193], f32, isOutput=False)
    out_d = nc.declare_dram_parameter("logT", [V, BS], f32, isOutput=True)

    with tile.TileContext(nc) as tc:
        with (
            tc.tile_pool(name="sb", bufs=1) as pool,
            tc.tile_pool(name="ps", bufs=1, space="PSUM") as pp,
        ):
            wide = pool.tile([V, 9 * BS], f32)
            nc.sync.dma_start(wide[:], toks_d[:].to_broadcast((V, 9 * BS)))
            cst = pool.tile([H + 1, 193], f32)
            nc.sync.dma_start(cst[:], cst_d[:])
            iota = cst[0:V, 192:193]

            qT = pool.tile([V + 1, BS], f32)    # one-hot(query) + ones row
            nc.vector.memset(qT[V:V + 1, :], 1.0)
            hid = pool.tile([H + 1, BS], f32)   # relu(hidT) + ones row
            nc.vector.memset(hid[H:H + 1, :], 1.0)

            # one-hot masks: masks[v, p*BS+b] = (seqs[b, 55+p] == v)
            m8 = pool.tile([V, NPOS * BS], bf16)
            nc.vector.tensor_scalar(m8[:], wide[:, 0:NPOS * BS], iota, None, eq)
            t1 = pool.tile([V, 4 * BS], bf16)
            nc.vector.tensor_add(t1[:], m8[:, 0:4 * BS], m8[:, 4 * BS:8 * BS])
            t2 = pool.tile([V, 2 * BS], bf16)
            nc.vector.tensor_add(t2[:], t1[:, 0:2 * BS], t1[:, 2 * BS:4 * BS])
            histT = pool.tile([V, BS], f32)
            nc.vector.tensor_add(histT[:], t2[:, 0:BS], t2[:, BS:2 * BS])
            nc.vector.tensor_scalar(qT[0:V, :], wide[:, NPOS * BS:9 * BS], iota, None, eq)

            # hidT = B^T @ histT + [A; r1b]^T @ [qT; 1]  (biases folded via K=65)
            hidT_ps = pp.tile([H, BS], f32, tag="hid")
            nc.tensor.matmul(hidT_ps[:], cst[0:V, 64:128], histT[:], start=True, stop=False)
            nc.tensor.matmul(hidT_ps[:], cst[0:V + 1, 0:64], qT[:], start=False, stop=True)
            nc.vector.tensor_scalar_max(hid[0:H, :], hidT_ps[:], 0.0)

            # logT = [r2wT; r2b]^T @ [hid; 1]
            logT_ps = pp.tile([V, BS], f32, tag="log")
            nc.tensor.matmul(logT_ps[:], cst[0:H + 1, 128:192], hid[:], start=True, stop=True)
            logT_sb = pool.tile([V, BS], f32)
            nc.vector.tensor_copy(logT_sb[:], logT_ps[:])
            nc.sync.dma_start(out_d[:], logT_sb[:])

    nc.compile()
    return nc


def _prep_in_maps(inputs):
    embed = np.asarray(inputs["embed"], dtype=np.float32)[:V]      # (64, 64)
    r1_w = np.asarray(inputs["r1_w"], dtype=np.float32)            # (64, 128)
    r1_b = np.asarray(inputs["r1_b"], dtype=np.float32)            # (64,)
    r2_w = np.asarray(inputs["r2_w"], dtype=np.float32)            # (64, 64)
    r2_b = np.asarray(inputs["r2_b"], dtype=np.float32)            # (64,)
    seqs = np.asarray(inputs["seqs"])                              # (B, 64) int
    query = np.asarray(inputs["query_tok"])                        # (B,) int

    A = embed @ r1_w[:, :H].T                                      # (64v, 64h)
    Bm = (embed @ r1_w[:, H:].T) * np.float32(1.0 / NPOS)          # (64v, 64h)
    cst = np.zeros((H + 1, 193), np.float32)
    cst[0:V, 0:64] = A
    cst[V, 0:64] = r1_b
    cst[0:V, 64:128] = Bm
    cst[0:H, 128:192] = r2_w.T
    cst[H, 128:192] = r2_b
    cst[0:V, 192] = np.arange(V, dtype=np.float32)

    # token rows, position-major: toks[p*BS + b] for shard rows b
    toks = np.empty((N_CORES, 9, BS), np.float32)
    tail = seqs[:, TAIL_LO:TAIL_HI].astype(np.float32)             # (B, 8)
    toks[:, :NPOS, :] = tail.reshape(N_CORES, BS, NPOS).transpose(0, 2, 1)
    toks[:, NPOS, :] = query.astype(np.float32).reshape(N_CORES, BS)

    return [
        {"toks": toks[c].reshape(1, 9 * BS), "cst": cst}
        for c in range(N_CORES)
    ]


def kernel(**inputs):
    global _compiled_nc
    from concourse.bass_utils import run_bass_kernel_spmd

    in_maps = _prep_in_maps(inputs)
    if _compiled_nc is None:
        _compiled_nc = _build_program()
    res = run_bass_kernel_spmd(_compiled_nc, in_maps, list(range(N_CORES)))
    out = np.empty((B, V), np.float32)
    for c in range(N_CORES):
        out[c * BS:(c + 1) * BS] = res.results[c]["logT"].T
    return out


if __name__ == "__main__":
    rng = np.random.default_rng(0)
    demo = {
        "embed": rng.standard_normal((V + 2, H)).astype(np.float32),
        "r1_w": rng.standard_normal((H, 2 * H)).astype(np.float32) * 0.05,
        "r1_b": rng.standard_normal(H).astype(np.float32) * 0.02,
        "r2_w": rng.standard_normal((V, H)).astype(np.float32) * 0.05,
        "r2_b": rng.standard_normal(V).astype(np.float32) * 0.02,
        "seqs": rng.integers(0, V, (B, 64)),
        "query_tok": rng.integers(0, V, (B,)),
    }
    out = kernel(**demo)
    tail = demo["embed"][demo["seqs"][:, TAIL_LO:TAIL_HI]]
    mem = tail.sum(1) / NPOS
    h = np.concatenate([demo["embed"][demo["query_tok"]], mem], -1)
    exp = np.maximum(h @ demo["r1_w"].T + demo["r1_b"], 0) @ demo["r2_w"].T + demo["r2_b"]
    err = np.abs(out - exp).max() / np.abs(exp).max()
    print("self-check rel err:", err)


# revision 7
# speedup vs baseline: 1.6962x; 1.1054x over previous
"""Trainium2 Bass kernel for nn_ConsolidationModel.

Mathematical reduction (verified bit-exact against the reference scan):
the scan's control flow is data-independent (count depends only on t).
Consolidation fires at t=15/31/47, but between consecutive firings the
8-slot FIFO receives 4 appends + 12 shift-appends, which evicts every
consolidated row before the next firing — and after the last firing
(t=47) there are 4 appends + 11 shifts, so at t=62 the buffer holds
exactly the embeddings of tokens 55..62 with count=8.  The model output
is therefore:

    mem  = mean_p embed[seqs[:, 55+p]]          (p = 0..7)
    h    = concat([embed[query_tok], mem], -1)  (B, 128)
    out  = relu(h @ r1_w.T + r1_b) @ r2_w.T + r2_b

Device algorithm (per core, batch shard of 256 rows; vocab/hidden on
partitions, batch on the free axis — 11 instructions total):

  wide (64, 2304) i16 <- ONE DMA with a stride-0 partition AP: the 9
                         token rows (8 tail positions + query)
                         broadcast across the 64 vocab partitions
  m8    = (wide[:, :2048] == iota)  bf16 one-hot masks        (1 DVE op)
  histT = bf16 add-tree over the 8 position masks             (3 DVE ops)
  qT    = (wide[:, 2048:] == iota)  f32                       (1 DVE op)
  hidT  = [B; A]^T @ [histT; qT]   ONE K=128 matmul, where
          A = embed @ r1_w[:, :64].T, B = embed @ r1_w[:, 64:].T / 8
          are folded on the host (data-independent weight prep)
  hid   = relu(hidT + r1_b)        one 2-op tensor_scalar
  logT  = r2_w.T^T @ hid           K=64 matmul
  out   = logT + r2_b              tensor_scalar add (PSUM -> SBUF)
  -> DMA out (64, 256); the host transposes each shard back to
     (256, 64) while gathering the 8 shards.

Sharding: pure data parallel over batch across 8 cores; parameters
replicated.
"""

import numpy as np

N_CORES = 8
B = 2048           # full batch
BS = B // N_CORES  # 256 per-core batch shard
H = 64             # hidden dim
V = 64             # vocab
TAIL_LO, TAIL_HI = 55, 63  # token positions that survive in the buffer
NPOS = TAIL_HI - TAIL_LO   # 8

_compiled_nc = None


def _build_program():
    import concourse.bacc as bacc
    import concourse.mybir as mybir
    from concourse import tile

    f32 = mybir.dt.float32
    bf16 = mybir.dt.bfloat16
    i16 = mybir.dt.int16
    eq = mybir.AluOpType.is_equal
    add = mybir.AluOpType.add
    mx = mybir.AluOpType.max

    nc = bacc.Bacc("TRN2", target_bir_lowering=False, debug=False,
                   num_devices=N_CORES)

    toks_d = nc.declare_dram_parameter("toks", [1, 9 * BS], i16, isOutput=False)
    cst_d = nc.declare_dram_parameter("cst", [2 * H, 131], f32, isOutput=False)
    out_d = nc.declare_dram_parameter("logT", [V, BS], f32, isOutput=True)

    with tile.TileContext(nc) as tc:
        with (
            tc.tile_pool(name="sb", bufs=1) as pool,
            tc.tile_pool(name="ps", bufs=1, space="PSUM") as pp,
        ):
            wide = pool.tile([V, 9 * BS], i16)
            nc.sync.dma_start(wide[:], toks_d[:].to_broadcast((V, 9 * BS)))
            cst = pool.tile([2 * H, 131], f32)
            nc.sync.dma_start(cst[:], cst_d[:])
            iota = cst[0:V, 130:131]
            r1b = cst[0:H, 128:129]
            r2b = cst[0:V, 129:130]

            # one-hot masks + histogram (bf16 add tree; counts <= 8 exact)
            hq = pool.tile([2 * H, BS], f32)   # rows 0:64 histT, 64:128 qT
            m8 = pool.tile([V, NPOS * BS], bf16)
            nc.vector.tensor_scalar(m8[:], wide[:, 0:NPOS * BS], iota, None, eq)
            t1 = pool.tile([V, 4 * BS], bf16)
            nc.vector.tensor_add(t1[:], m8[:, 0:4 * BS], m8[:, 4 * BS:8 * BS])
            t2 = pool.tile([V, 2 * BS], bf16)
            nc.vector.tensor_add(t2[:], t1[:, 0:2 * BS], t1[:, 2 * BS:4 * BS])
            nc.vector.tensor_add(hq[0:V, :], t2[:, 0:BS], t2[:, BS:2 * BS])
            nc.vector.tensor_scalar(hq[V:2 * V, :], wide[:, NPOS * BS:9 * BS], iota, None, eq)

            # hidT = B^T @ histT + A^T @ qT   (single K=128 matmul)
            hidT_ps = pp.tile([H, BS], f32, tag="hid")
            nc.tensor.matmul(hidT_ps[:], cst[:, 0:64], hq[:], start=True, stop=True)
            # hid = relu(hidT + r1_b)
            hid = pool.tile([H, BS], f32)
            nc.vector.tensor_scalar(hid[:], hidT_ps[:], r1b, 0.0, add, mx)

            # logT = r2wT^T @ hid ; + r2_b on the PSUM->SBUF move
            logT_ps = pp.tile([V, BS], f32, tag="log")
            nc.tensor.matmul(logT_ps[:], cst[0:H, 64:128], hid[:], start=True, stop=True)
            logT_sb = pool.tile([V, BS], f32)
            nc.vector.tensor_scalar(logT_sb[:], logT_ps[:], r2b, None, add)
            nc.sync.dma_start(out_d[:], logT_sb[:])

    nc.compile()
    return nc


def _prep_in_maps(inputs):
    embed = np.asarray(inputs["embed"], dtype=np.float32)[:V]      # (64, 64)
    r1_w = np.asarray(inputs["r1_w"], dtype=np.float32)            # (64, 128)
    r1_b = np.asarray(inputs["r1_b"], dtype=np.float32)            # (64,)
    r2_w = np.asarray(inputs["r2_w"], dtype=np.float32)            # (64, 64)
    r2_b = np.asarray(inputs["r2_b"], dtype=np.float32)            # (64,)
    seqs = np.asarray(inputs["seqs"])                              # (B, 64) int
    query = np.asarray(inputs["query_tok"])                        # (B,) int

    A = embed @ r1_w[:, :H].T                                      # (64v, 64h)
    Bm = (embed @ r1_w[:, H:].T) * np.float32(1.0 / NPOS)          # (64v, 64h)
    cst = np.zeros((2 * H, 131), np.float32)
    cst[0:V, 0:64] = Bm
    cst[V:2 * V, 0:64] = A
    cst[0:H, 64:128] = r2_w.T
    cst[0:H, 128] = r1_b
    cst[0:V, 129] = r2_b
    cst[0:V, 130] = np.arange(V, dtype=np.float32)

    # token rows, position-major: toks[p*BS + b] for shard rows b
    toks = np.empty((N_CORES, 9, BS), np.int16)
    toks[:, :NPOS, :] = (
        seqs[:, TAIL_LO:TAIL_HI].astype(np.int16).reshape(N_CORES, BS, NPOS)
        .transpose(0, 2, 1))
    toks[:, NPOS, :] = query.astype(np.int16).reshape(N_CORES, BS)

    return [
        {"toks": toks[c].reshape(1, 9 * BS), "cst": cst}
        for c in range(N_CORES)
    ]


def kernel(**inputs):
    global _compiled_nc
    from concourse.bass_utils import run_bass_kernel_spmd

    in_maps = _prep_in_maps(inputs)
    if _compiled_nc is None:
        _compiled_nc = _build_program()
    res = run_bass_kernel_spmd(_compiled_nc, in_maps, list(range(N_CORES)))
    out = np.empty((B, V), np.float32)
    for c in range(N_CORES):
        out[c * BS:(c + 1) * BS] = res.results[c]["logT"].T
    return out


if __name__ == "__main__":
    rng = np.random.default_rng(0)
    demo = {
        "embed": rng.standard_normal((V + 2, H)).astype(np.float32),
        "r1_w": rng.standard_normal((H, 2 * H)).astype(np.float32) * 0.05,
        "r1_b": rng.standard_normal(H).astype(np.float32) * 0.02,
        "r2_w": rng.standard_normal((V, H)).astype(np.float32) * 0.05,
        "r2_b": rng.standard_normal(V).astype(np.float32) * 0.02,
        "seqs": rng.integers(0, V, (B, 64)),
        "query_tok": rng.integers(0, V, (B,)),
    }
    out = kernel(**demo)
    tail = demo["embed"][demo["seqs"][:, TAIL_LO:TAIL_HI]]
    mem = tail.sum(1) / NPOS
    h = np.concatenate([demo["embed"][demo["query_tok"]], mem], -1)
    exp = np.maximum(h @ demo["r1_w"].T + demo["r1_b"], 0) @ demo["r2_w"].T + demo["r2_b"]
    err = np.abs(out - exp).max() / np.abs(exp).max()
    print("self-check rel err:", err)
